# revision 34
# baseline (speedup 1.0000x reference)
"""Trainium2 Bass kernel for nn_CESAR_24309514895978 (ragged_sequence).

Math (per batch b):
  m0 = (am==1)&(tt==0); m1 = (am==1)&(tt==1)
  score[i,j] = |emb_n[i] . emb_n[j]|   (L2-normalized embeddings)
  logits[i,j] = (emb@Wq.T+bq)[i] . (emb@Wk.T+bk)[j]
  cs[b] = sum_{valid ij} softmax_flat(logits | i in m0, j in m1)[i,j] * score[i,j]

Ragged compaction: only ~25% of tokens are in m0 and ~25% in m1, so the
host gathers valid tokens and the device works on compacted panels:
rows = m1 tokens of 2 batches packed (<=2*128 partitions), cols = m0
tokens in the free dim (~260).  Matmul cost scales with the free dim
only, so rows use the partition dim and cols the free dim; batches are
paired to cores so both fit.

Constant folding (host): logits = embaug_r @ A_aug @ embaug_c.T with
A_aug = [[Wk.T@Wq, Wk.T@bq], [bk.T@Wq, bq.bk]].  The device gets
  at  = (Wq.T@Wk)[db, da]      stage-1 lhsT, fp16
  ucol= Wk.T@bq                bias riding the PSUM->SBUF copy of P
  prow= emb_c@(Wq.T@bk)+bq.bk  host-computed rank-1 row (mask matmul)
Norms r=1/||emb|| are host-computed; r_c rides a broadcast row, r_r is
applied host-side to the W partials.

Batch identity inside a packed panel comes from a K=4 fp16 mask matmul
(sum of non-positive rank-1 terms, -60000 each, so exp underflows to
exactly 0 like the reference's -1e30 fill):
  [ones, b0r, b1r, padr] x [prow-60000*padc, -6e4*b1c, -6e4*b0c, -6e4*ones]
No on-device max: exp uses bias -M0 (an uploaded [128,1] column, so a
retry with larger M0 needs no recompile); W/Z ratio cancels the shift.

Perf notes (from perfetto traces):
  - each dma_start costs ~0.65us on its issuing sequencer: spread the
    triggers across Sync/Scalar/GpSimd in data-need order, consolidate
    small tensors, and fetch wide lines (2KB+/partition descriptors).
  - tile-granularity deps: emb panels are split into a/b half tiles so
    stage 1 starts after half the panel (+ at[0]) lands.
  - the PE runs at half clock for its first ~3us (p-state ramp) and the
    ramp restarts after idle gaps: dummy warmup matmuls ramp it while
    the first DMAs land, and the trigger order keeps stage 1 gapless.
Device per core: stage1 P = at.T @ embc (64 mm), gram G = embr.T @ embc
(16 mm), stage2 L = embr.T @ paug + mask (18 mm); exp+accum -> Z rows,
stt(gw,E)+accum -> W rows.  Host: segment-sum rows by batch, cs = W/Z.
"""
import numpy as np

import concourse.tile as tile
from concourse import bacc, mybir
from concourse.bass_utils import run_bass_kernel_spmd

B, S, D = 16, 512, 1024
NCORES = 8
NCH = D // 128             # 8 contraction chunks
NEGH = np.float32(-60000.0)  # fp16-safe mask fill; exp(x-60000) == 0 exactly
M0 = 60.0                  # logit shift; exp(L - M0), max logit ~58
EPS = 1e-12

F32 = mybir.dt.float32
F16 = mybir.dt.float16     # 10-bit mantissa: full-rate PE, 2B/elem
AFT = mybir.ActivationFunctionType
ALU = mybir.AluOpType

PROFILE = False            # set True (e.g. from test.py) to capture NTFF profile
LAST_RESULTS = None        # BassKernelResults of the last run (for test.py)

_builds = {}


def _f16(x: np.ndarray) -> np.ndarray:
    return np.ascontiguousarray(np.asarray(x, np.float32)).astype(np.float16)


def _build(nr: int, c: int):
    key = (nr, c)
    if key in _builds:
        return _builds[key]

    R = nr * 128
    hc = (NCH // 2) * c     # half-panel widths
    hr = (NCH // 2) * R
    nc = bacc.Bacc("TRN2", target_bir_lowering=False, debug=False)

    at_d = nc.dram_tensor("at", [NCH, 128, D], F16, kind="ExternalInput").ap()
    embc_d = nc.dram_tensor("embc", [128, NCH * c], F16, kind="ExternalInput").ap()
    embr_d = nc.dram_tensor("embr", [128, NCH * R], F16, kind="ExternalInput").ap()
    # mr4 cols 0:R: mask lhsT; cols R:R+c: mask rhs (same base partition)
    mr4_d = nc.dram_tensor("mr4", [4, R + c], F16, kind="ExternalInput").ap()
    # ucm cols 0-7: ucol (P chunk bias); col 8: -M0 exp bias
    ucm_d = nc.dram_tensor("ucm", [128, NCH + 1], F32, kind="ExternalInput").ap()
    rrow_d = nc.dram_tensor("rrow", [1, c], F32, kind="ExternalInput").ap()

    zw_d = nc.dram_tensor("zw", [128, 2 * nr], F32, kind="ExternalOutput").ap()

    with tile.TileContext(nc) as tc:
        with (
            tc.tile_pool(name="apool", bufs=NCH + 1) as apool,
            tc.tile_pool(name="cpool", bufs=4) as cpool,
            tc.tile_pool(name="rpool", bufs=2) as rpool,
            tc.tile_pool(name="paugpool", bufs=NCH) as paugpool,
            tc.tile_pool(name="gapool", bufs=2) as gapool,
            tc.tile_pool(name="gwpool", bufs=2) as gwpool,
            tc.tile_pool(name="Epool", bufs=2) as Epool,
            tc.tile_pool(name="scrpool", bufs=2) as scrpool,
            tc.tile_pool(name="w2pool", bufs=1) as w2pool,
            tc.tile_pool(name="tiny", bufs=6) as tiny,
            tc.tile_pool(name="ps", bufs=8, space="PSUM") as ps,
        ):
            at_t = [None] + [apool.tile([128, D], F16, tag="a", name=f"at{k}")
                             for k in range(1, NCH)]
            at0h = [apool.tile([128, D // 2], F16, tag="a0", name=f"at0{h}")
                    for h in range(2)]
            qc = 2 * c
            embc_q = [cpool.tile([128, qc], F16, tag="c", name=f"embc{h}")
                      for h in range(4)]
            embr_ab = [rpool.tile([128, hr], F16, tag="r", name=f"embr{h}")
                       for h in range(2)]
            mr4_t = tiny.tile([4, R + c], F16, tag="mr4")
            ucm_t = tiny.tile([128, NCH + 1], F32, tag="ucm")
            rrow_t = tiny.tile([1, c], F32, tag="rr")
            warm_t = tiny.tile([128, 128], F16, tag="warm")

            nc.vector.memset(warm_t[:], 0.0)

            # ---- DMA triggers (~0.65us each on a sequencer): three engines
            # issue in parallel, ordered by when the PE needs the data.
            nc.sync.dma_start(out=embc_q[0][:], in_=embc_d[:, 0:qc])
            nc.scalar.dma_start(out=at0h[0][:], in_=at_d[0, :, 0:D // 2])
            nc.scalar.dma_start(out=at0h[1][:], in_=at_d[0, :, D // 2:])
            nc.sync.dma_start(out=at_t[1][:], in_=at_d[1])
            nc.gpsimd.dma_start(out=embc_q[1][:], in_=embc_d[:, qc:2 * qc])
            nc.gpsimd.dma_start(out=at_t[2][:], in_=at_d[2])
            nc.sync.dma_start(out=at_t[3][:], in_=at_d[3])
            nc.scalar.dma_start(out=embc_q[2][:], in_=embc_d[:, 2 * qc:3 * qc])
            nc.gpsimd.dma_start(out=at_t[4][:], in_=at_d[4])
            nc.sync.dma_start(out=at_t[5][:], in_=at_d[5])
            nc.scalar.dma_start(out=embc_q[3][:], in_=embc_d[:, 3 * qc:])
            nc.gpsimd.dma_start(out=at_t[6][:], in_=at_d[6])
            nc.sync.dma_start(out=at_t[7][:], in_=at_d[7])
            nc.scalar.dma_start(out=embr_ab[0][:], in_=embr_d[:, 0:hr])
            nc.gpsimd.dma_start(out=embr_ab[1][:], in_=embr_d[:, hr:])
            nc.sync.dma_start(out=mr4_t[:], in_=mr4_d)
            nc.scalar.dma_start(out=ucm_t[:], in_=ucm_d)
            nc.gpsimd.dma_start(out=rrow_t[:], in_=rrow_d)

            # ---- PE warmup: ramp the clock out of low p-state while the
            # first input DMAs land; results are discarded.
            warm_ps = ps.tile([128, 512], F32, tag="ps", name="warm_ps")
            for _ in range(34):
                nc.tensor.matmul(warm_ps[:, 0:128], warm_t[:], warm_t[:],
                                 start=True, stop=True)

            # ---- stage 1: P = at.T @ embc  (db-outer over 8 banks)
            st1 = [ps.tile([128, 512], F32, tag="ps", name=f"st1_{da}")
                   for da in range(NCH)]
            for db in range(NCH):
                rhs = embc_q[db // 2][:, (db % 2) * c:(db % 2 + 1) * c]
                for da in range(NCH):
                    if db == 0:
                        lhs = at0h[da // 4][:, (da % 4) * 128:(da % 4 + 1) * 128]
                    else:
                        lhs = at_t[db][:, da * 128:(da + 1) * 128]
                    nc.tensor.matmul(st1[da][:, 0:c], lhs, rhs,
                                     start=(db == 0), stop=(db == NCH - 1))
            # PSUM -> SBUF with the u-column bias, split across ACT/DVE
            paug = []
            for da in range(NCH):
                pt = paugpool.tile([128, c], F16, tag="paug")
                if da % 2 == 0:
                    nc.scalar.activation(out=pt[:], in_=st1[da][:, 0:c],
                                         func=AFT.Identity,
                                         bias=ucm_t[:, da:da + 1], scale=1.0)
                else:
                    nc.vector.tensor_scalar_add(pt[:], st1[da][:, 0:c],
                                                ucm_t[:, da:da + 1])
                paug.append(pt)

            # ---- W2 = broadcast of r over cols
            W2 = w2pool.tile([128, c], F32, tag="w2")
            nc.gpsimd.partition_broadcast(W2[:], rrow_t[0:1, :], channels=128)

            # ---- gram -> gw = |G| * r_c  (overlaps the paug copies)
            gw_t = []
            for yc in range(nr):
                Gp = ps.tile([128, 512], F32, tag="ps", name=f"G_{yc}")
                for d2 in range(NCH):
                    lhs = embr_ab[d2 // 4][:, (d2 % 4) * R + yc * 128:
                                           (d2 % 4) * R + (yc + 1) * 128]
                    nc.tensor.matmul(Gp[:, 0:c], lhs,
                                     embc_q[d2 // 2][:, (d2 % 2) * c:
                                                     (d2 % 2 + 1) * c],
                                     start=(d2 == 0), stop=(d2 == NCH - 1))
                ga = gapool.tile([128, c], F32, tag="ga")
                nc.scalar.activation(out=ga[:], in_=Gp[:, 0:c], func=AFT.Abs,
                                     bias=0.0, scale=1.0)
                gw = gwpool.tile([128, c], F32, tag="gw")
                nc.vector.tensor_mul(gw[:], ga[:], W2[:])
                gw_t.append(gw)

            # ---- stage 2: L = mask + embr.T @ paug; exp/stt with accums
            ztile = tiny.tile([128, nr], F32, tag="z")
            wtile = tiny.tile([128, nr], F32, tag="w")
            for yc in range(nr):
                Lp = ps.tile([128, 512], F32, tag="ps", name=f"L_{yc}")
                nc.tensor.matmul(Lp[:, 0:c], mr4_t[:, yc * 128:(yc + 1) * 128],
                                 mr4_t[:, R:R + c], start=True, stop=False)
                for da in range(NCH):
                    lhs = embr_ab[da // 4][:, (da % 4) * R + yc * 128:
                                           (da % 4) * R + (yc + 1) * 128]
                    nc.tensor.matmul(Lp[:, 0:c], lhs, paug[da][:],
                                     start=False, stop=(da == NCH - 1))
                E = Epool.tile([128, c], F32, tag="E")
                nc.scalar.activation(out=E[:], in_=Lp[:, 0:c], func=AFT.Exp,
                                     bias=ucm_t[:, NCH:NCH + 1], scale=1.0,
                                     accum_out=ztile[:, yc:yc + 1])
                scr = scrpool.tile([128, c], F32, tag="scr")
                nc.vector.scalar_tensor_tensor(
                    out=scr[:], in0=gw_t[yc][:], scalar=1.0, in1=E[:],
                    op0=ALU.mult, op1=ALU.mult,
                    accum_out=wtile[:, yc:yc + 1])

            nc.scalar.dma_start(out=zw_d[:, 0:nr], in_=ztile[:])
            nc.gpsimd.dma_start(out=zw_d[:, nr:2 * nr], in_=wtile[:])
    nc.compile()
    _builds[key] = nc
    return nc


def _pick_pairing(n_rows: np.ndarray, n_cols: np.ndarray, r_cap: int = 256):
    """Pair 16 batches into 8 cores: row sums (m1) must fit r_cap, minimize
    the max col sum (m0) which sets the matmul free dim."""
    order = np.argsort(-n_rows, kind="stable")
    pairs = [[int(order[k]), int(order[B - 1 - k])] for k in range(B // 2)]

    def rsum(p):
        return n_rows[p[0]] + n_rows[p[1]]

    def csum(p):
        return n_cols[p[0]] + n_cols[p[1]]

    # 2-opt: shrink max col sum with swaps that keep rows under the cap
    for _ in range(64):
        worst = max(range(len(pairs)), key=lambda i: csum(pairs[i]))
        best = None
        for j in range(len(pairs)):
            if j == worst:
                continue
            for a in range(2):
                for bidx in range(2):
                    p1 = list(pairs[worst])
                    p2 = list(pairs[j])
                    p1[a], p2[bidx] = p2[bidx], p1[a]
                    if rsum(p1) > r_cap or rsum(p2) > r_cap:
                        continue
                    new_max = max(csum(p1), csum(p2))
                    if new_max < csum(pairs[worst]):
                        if best is None or new_max < best[0]:
                            best = (new_max, j, a, bidx)
        if best is None:
            break
        _, j, a, bidx = best
        pairs[worst][a], pairs[j][bidx] = pairs[j][bidx], pairs[worst][a]

    rmax = max(rsum(p) for p in pairs)
    cmax = max(csum(p) for p in pairs)
    nr = int(np.ceil(max(rmax, 1) / 128))
    c = max(144, -(-int(max(cmax, 1)) // 16) * 16)
    return nr, c, [tuple(p) for p in pairs]


def kernel(embeddings, Wq, bq, Wk, bk, attention_masks, token_type_ids):
    global LAST_RESULTS

    emb = np.ascontiguousarray(np.asarray(embeddings, dtype=np.float32))
    Wq = np.asarray(Wq, dtype=np.float32)
    Wk = np.asarray(Wk, dtype=np.float32)
    bq = np.asarray(bq, dtype=np.float32)
    bk = np.asarray(bk, dtype=np.float32)
    am = np.asarray(attention_masks)
    tt = np.asarray(token_type_ids)

    tok = am == 1
    m0 = tok & (tt == 0)   # cols
    m1 = tok & (tt == 1)   # rows
    n_cols = m0.sum(1)
    n_rows = m1.sum(1)

    nr, c, pairing = _pick_pairing(n_rows, n_cols)
    R = nr * 128
    nc = _build(nr, c)

    # ---- host constant folding
    Wq64, Wk64 = Wq.astype(np.float64), Wk.astype(np.float64)
    A = (Wq64.T @ Wk64).astype(np.float32)          # [db, da] stage-1 lhsT
    at16 = _f16(A).reshape(NCH, 128, D)
    u = (Wk64.T @ bq.astype(np.float64)).astype(np.float32)       # P bias
    ucm = np.empty((128, NCH + 1), np.float32)
    ucm[:, 0:NCH] = u.reshape(NCH, 128).T
    ucm[:, NCH] = -M0
    u2 = Wq64.T @ bk.astype(np.float64)             # prow direction
    c0 = float(bq.astype(np.float64) @ bk.astype(np.float64))

    nrm = np.sqrt(np.einsum("bsd,bsd->bs", emb, emb, dtype=np.float64))
    rr_full = (1.0 / np.maximum(nrm, EPS)).astype(np.float32)     # [B, S]

    in_maps = []
    row_meta = []
    for (b0, b1) in pairing:
        ridx = [(b, j) for b in (b0, b1) for j in np.nonzero(m1[b])[0]]
        cidx = [(b, j) for b in (b0, b1) for j in np.nonzero(m0[b])[0]]
        nrow0 = int(n_rows[b0])
        ncol0 = int(n_cols[b0])
        nrow = len(ridx)
        ncol = len(cidx)

        er = np.zeros((R, D), np.float32)
        for i, (b, j) in enumerate(ridx):
            er[i] = emb[b, j]
        ec = np.zeros((c, D), np.float32)
        for i, (b, j) in enumerate(cidx):
            ec[i] = emb[b, j]

        # pack [tok, D] -> [128, NCH*n]: chunk k at cols [k*n, (k+1)*n),
        # partition p <-> d = k*128+p
        erw = er.T.reshape(NCH, 128, R).transpose(1, 0, 2).reshape(128, NCH * R)
        ecw = ec.T.reshape(NCH, 128, c).transpose(1, 0, 2).reshape(128, NCH * c)

        prow = (ec.astype(np.float64) @ u2 + c0).astype(np.float32)
        prow[ncol:] = NEGH                      # padded cols masked via row0

        mr4 = np.zeros((4, R + c), np.float32)
        mr4[0, :R] = 1.0                        # ones row (prow + pad-col term)
        mr4[1, :nrow0] = 1.0                    # b0 rows
        mr4[2, nrow0:nrow] = 1.0                # b1 rows
        mr4[3, nrow:R] = 1.0                    # padded rows
        mr4[0, R:] = prow
        mr4[1, R + ncol0:R + ncol] = NEGH       # b1 cols, masked for b0 rows
        mr4[2, R:R + ncol0] = NEGH              # b0 cols, masked for b1 rows
        mr4[3, R:] = NEGH                       # all cols, masked for pad rows

        rrow = np.zeros((1, c), np.float32)
        rrow[0, :ncol] = [rr_full[b, j] for (b, j) in cidx]

        in_maps.append({
            "at": at16,
            "embc": _f16(ecw),
            "embr": _f16(erw),
            "mr4": _f16(mr4),
            "ucm": ucm,
            "rrow": rrow,
        })
        row_meta.append((b0, nrow0, b1, nrow - nrow0, ridx))

    valid = m0.any(axis=1) & m1.any(axis=1)
    for attempt in range(3):
        res = run_bass_kernel_spmd(nc, in_maps, core_ids=list(range(NCORES)),
                                   trace=PROFILE)
        LAST_RESULTS = res
        ok = all(np.isfinite(res.results[i]["zw"]).all() for i in range(NCORES))
        if ok:
            break
        for im in in_maps:    # overflow escape hatch: larger shift, no recompile
            im["ucm"] = im["ucm"].copy()
            im["ucm"][:, NCH] *= 4.0

    cs = np.zeros(B, np.float64)
    for i in range(NCORES):
        zw = res.results[i]["zw"].astype(np.float64)      # [128, 2*nr]
        zflat = zw[:, 0:nr].T.ravel()                     # row-major [R]
        wflat = zw[:, nr:2 * nr].T.ravel()
        b0, nrow0, b1, nrow1, ridx = row_meta[i]
        r_rows = np.zeros(R, np.float64)
        r_rows[:len(ridx)] = [rr_full[b, j] for (b, j) in ridx]
        wr = wflat * r_rows
        if valid[b0]:
            cs[b0] = wr[:nrow0].sum() / (zflat[:nrow0].sum() + 1e-300)
        if valid[b1]:
            cs[b1] = (wr[nrow0:nrow0 + nrow1].sum()
                      / (zflat[nrow0:nrow0 + nrow1].sum() + 1e-300))
    return cs.astype(np.float32)


# revision 35
# speedup vs baseline: 1.0407x; 1.0407x over previous
"""Trainium2 Bass kernel for nn_CESAR_24309514895978 (ragged_sequence).

Math (per batch b):
  m0 = (am==1)&(tt==0); m1 = (am==1)&(tt==1)
  score[i,j] = |emb_n[i] . emb_n[j]|   (L2-normalized embeddings)
  logits[i,j] = (emb@Wq.T+bq)[i] . (emb@Wk.T+bk)[j]
  cs[b] = sum_{valid ij} softmax_flat(logits | i in m0, j in m1)[i,j] * score[i,j]

Ragged compaction: only ~25% of tokens are in m0 and ~25% in m1, so the
host gathers valid tokens and the device works on compacted panels:
rows = m1 tokens of 2 batches packed (<=2*128 partitions), cols = m0
tokens in the free dim (~260).  Matmul cost scales with the free dim
only, so rows use the partition dim and cols the free dim; batches are
paired to cores so both fit.

Constant folding (host): logits = embaug_r @ A_aug @ embaug_c.T with
A_aug = [[Wk.T@Wq, Wk.T@bq], [bk.T@Wq, bq.bk]].  The device gets
  at  = (Wq.T@Wk)[db, da]      stage-1 lhsT, fp16
  ucol= Wk.T@bq                bias riding the PSUM->SBUF copy of P
  prow= emb_c@(Wq.T@bk)+bq.bk  host-computed rank-1 row (mask matmul)
Norms r=1/||emb|| are host-computed; r_c rides a broadcast row, r_r is
applied host-side to the W partials.

Batch identity inside a packed panel comes from a K=4 fp16 mask matmul
(sum of non-positive rank-1 terms, -60000 each, so exp underflows to
exactly 0 like the reference's -1e30 fill):
  [ones, b0r, b1r, padr] x [prow-60000*padc, -6e4*b1c, -6e4*b0c, -6e4*ones]
No on-device max: exp uses bias -M0 (an uploaded [128,1] column, so a
retry with larger M0 needs no recompile); W/Z ratio cancels the shift.

Perf notes (from perfetto traces):
  - each dma_start costs ~0.65us on its issuing sequencer: spread the
    triggers across Sync/Scalar/GpSimd in data-need order, consolidate
    small tensors, and fetch wide lines (2KB+/partition descriptors).
  - tile-granularity deps: emb panels are split into a/b half tiles so
    stage 1 starts after half the panel (+ at[0]) lands.
  - the PE runs at half clock for its first ~3us (p-state ramp) and the
    ramp restarts after idle gaps: dummy warmup matmuls ramp it while
    the first DMAs land, and the trigger order keeps stage 1 gapless.
Device per core: stage1 P = at.T @ embc (64 mm), gram G = embr.T @ embc
(16 mm), stage2 L = embr.T @ paug + mask (18 mm); exp+accum -> Z rows,
stt(gw,E)+accum -> W rows.  Host: segment-sum rows by batch, cs = W/Z.
"""
import numpy as np

import concourse.tile as tile
from concourse import bacc, mybir
from concourse.bass_utils import run_bass_kernel_spmd

B, S, D = 16, 512, 1024
NCORES = 8
NCH = D // 128             # 8 contraction chunks
NEGH = np.float32(-60000.0)  # fp16-safe mask fill; exp(x-60000) == 0 exactly
M0 = 60.0                  # logit shift; exp(L - M0), max logit ~58
EPS = 1e-12

F32 = mybir.dt.float32
F16 = mybir.dt.float16     # 10-bit mantissa: full-rate PE, 2B/elem
AFT = mybir.ActivationFunctionType
ALU = mybir.AluOpType

PROFILE = False            # set True (e.g. from test.py) to capture NTFF profile
LAST_RESULTS = None        # BassKernelResults of the last run (for test.py)

_builds = {}


def _f16(x: np.ndarray) -> np.ndarray:
    return np.ascontiguousarray(np.asarray(x, np.float32)).astype(np.float16)


def _build(nr: int, c: int):
    key = (nr, c)
    if key in _builds:
        return _builds[key]

    R = nr * 128
    hc = (NCH // 2) * c     # half-panel widths
    hr = (NCH // 2) * R
    nc = bacc.Bacc("TRN2", target_bir_lowering=False, debug=False)

    at_d = nc.dram_tensor("at", [NCH, 128, D], F16, kind="ExternalInput").ap()
    embc_d = nc.dram_tensor("embc", [128, NCH * c], F16, kind="ExternalInput").ap()
    embr_d = nc.dram_tensor("embr", [128, NCH * R], F16, kind="ExternalInput").ap()
    # mr4 cols 0:R: mask lhsT; cols R:R+c: mask rhs (same base partition)
    mr4_d = nc.dram_tensor("mr4", [4, R + c], F16, kind="ExternalInput").ap()
    # ucm cols 0-7: ucol (P chunk bias); col 8: -M0 exp bias
    ucm_d = nc.dram_tensor("ucm", [128, NCH + 1], F32, kind="ExternalInput").ap()
    rrow_d = nc.dram_tensor("rrow", [1, c], F32, kind="ExternalInput").ap()

    zw_d = nc.dram_tensor("zw", [128, 2 * nr], F32, kind="ExternalOutput").ap()

    with tile.TileContext(nc) as tc:
        with (
            tc.tile_pool(name="apool", bufs=NCH + 1) as apool,
            tc.tile_pool(name="cpool", bufs=4) as cpool,
            tc.tile_pool(name="rpool", bufs=2) as rpool,
            tc.tile_pool(name="paugpool", bufs=NCH) as paugpool,
            tc.tile_pool(name="gapool", bufs=2) as gapool,
            tc.tile_pool(name="gwpool", bufs=2) as gwpool,
            tc.tile_pool(name="Epool", bufs=2) as Epool,
            tc.tile_pool(name="scrpool", bufs=2) as scrpool,
            tc.tile_pool(name="w2pool", bufs=1) as w2pool,
            tc.tile_pool(name="tiny", bufs=6) as tiny,
            tc.tile_pool(name="ps", bufs=8, space="PSUM") as ps,
        ):
            at_t = [None] + [apool.tile([128, D], F16, tag="a", name=f"at{k}")
                             for k in range(1, NCH)]
            at0h = [apool.tile([128, D // 2], F16, tag="a0", name=f"at0{h}")
                    for h in range(2)]
            qc = 2 * c
            embc_q = [cpool.tile([128, qc], F16, tag="c", name=f"embc{h}")
                      for h in range(4)]
            embr_ab = [rpool.tile([128, hr], F16, tag="r", name=f"embr{h}")
                       for h in range(2)]
            mr4_t = tiny.tile([4, R + c], F16, tag="mr4")
            ucm_t = tiny.tile([128, NCH + 1], F32, tag="ucm")
            rrow_t = tiny.tile([1, c], F32, tag="rr")
            warm_t = tiny.tile([128, 128], F16, tag="warm")

            nc.vector.memset(warm_t[:], 0.0)

            # ---- DMA triggers (~0.65us each on a sequencer): three engines
            # issue in parallel, ordered by when the PE needs the data.
            nc.sync.dma_start(out=embc_q[0][:], in_=embc_d[:, 0:qc])
            nc.scalar.dma_start(out=at0h[0][:], in_=at_d[0, :, 0:D // 2])
            nc.scalar.dma_start(out=at0h[1][:], in_=at_d[0, :, D // 2:])
            nc.sync.dma_start(out=at_t[1][:], in_=at_d[1])
            nc.gpsimd.dma_start(out=embc_q[1][:], in_=embc_d[:, qc:2 * qc])
            nc.gpsimd.dma_start(out=at_t[2][:], in_=at_d[2])
            nc.sync.dma_start(out=at_t[3][:], in_=at_d[3])
            nc.scalar.dma_start(out=embc_q[2][:], in_=embc_d[:, 2 * qc:3 * qc])
            nc.gpsimd.dma_start(out=at_t[4][:], in_=at_d[4])
            nc.sync.dma_start(out=at_t[5][:], in_=at_d[5])
            nc.scalar.dma_start(out=embc_q[3][:], in_=embc_d[:, 3 * qc:])
            nc.gpsimd.dma_start(out=at_t[6][:], in_=at_d[6])
            nc.sync.dma_start(out=at_t[7][:], in_=at_d[7])
            nc.scalar.dma_start(out=embr_ab[0][:], in_=embr_d[:, 0:hr])
            nc.gpsimd.dma_start(out=embr_ab[1][:], in_=embr_d[:, hr:])
            nc.sync.dma_start(out=mr4_t[:], in_=mr4_d)
            nc.scalar.dma_start(out=ucm_t[:], in_=ucm_d)
            nc.gpsimd.dma_start(out=rrow_t[:], in_=rrow_d)

            # ---- PE warmup: ramp the clock out of low p-state while the
            # first input DMAs land; results are discarded.
            warm_ps = ps.tile([128, 512], F32, tag="ps", name="warm_ps")
            for _ in range(26):
                nc.tensor.matmul(warm_ps[:, 0:128], warm_t[:], warm_t[:],
                                 start=True, stop=True)

            # ---- stage 1: P = at.T @ embc  (db-outer over 8 banks)
            st1 = [ps.tile([128, 512], F32, tag="ps", name=f"st1_{da}")
                   for da in range(NCH)]
            for db in range(NCH):
                rhs = embc_q[db // 2][:, (db % 2) * c:(db % 2 + 1) * c]
                for da in range(NCH):
                    if db == 0:
                        lhs = at0h[da // 4][:, (da % 4) * 128:(da % 4 + 1) * 128]
                    else:
                        lhs = at_t[db][:, da * 128:(da + 1) * 128]
                    nc.tensor.matmul(st1[da][:, 0:c], lhs, rhs,
                                     start=(db == 0), stop=(db == NCH - 1))
            # PSUM -> SBUF with the u-column bias, split across ACT/DVE
            paug = []
            for da in range(NCH):
                pt = paugpool.tile([128, c], F16, tag="paug")
                if da % 2 == 0:
                    nc.scalar.activation(out=pt[:], in_=st1[da][:, 0:c],
                                         func=AFT.Identity,
                                         bias=ucm_t[:, da:da + 1], scale=1.0)
                else:
                    nc.vector.tensor_scalar_add(pt[:], st1[da][:, 0:c],
                                                ucm_t[:, da:da + 1])
                paug.append(pt)

            # ---- W2 = broadcast of r over cols
            W2 = w2pool.tile([128, c], F32, tag="w2")
            nc.gpsimd.partition_broadcast(W2[:], rrow_t[0:1, :], channels=128)

            # ---- gram -> gw = |G| * r_c  (overlaps the paug copies)
            gw_t = []
            for yc in range(nr):
                Gp = ps.tile([128, 512], F32, tag="ps", name=f"G_{yc}")
                for d2 in range(NCH):
                    lhs = embr_ab[d2 // 4][:, (d2 % 4) * R + yc * 128:
                                           (d2 % 4) * R + (yc + 1) * 128]
                    nc.tensor.matmul(Gp[:, 0:c], lhs,
                                     embc_q[d2 // 2][:, (d2 % 2) * c:
                                                     (d2 % 2 + 1) * c],
                                     start=(d2 == 0), stop=(d2 == NCH - 1))
                ga = gapool.tile([128, c], F32, tag="ga")
                nc.scalar.activation(out=ga[:], in_=Gp[:, 0:c], func=AFT.Abs,
                                     bias=0.0, scale=1.0)
                gw = gwpool.tile([128, c], F32, tag="gw")
                nc.vector.tensor_mul(gw[:], ga[:], W2[:])
                gw_t.append(gw)

            # ---- stage 2: L = mask + embr.T @ paug; exp/stt with accums
            ztile = tiny.tile([128, nr], F32, tag="z")
            wtile = tiny.tile([128, nr], F32, tag="w")
            for yc in range(nr):
                Lp = ps.tile([128, 512], F32, tag="ps", name=f"L_{yc}")
                nc.tensor.matmul(Lp[:, 0:c], mr4_t[:, yc * 128:(yc + 1) * 128],
                                 mr4_t[:, R:R + c], start=True, stop=False)
                for da in range(NCH):
                    lhs = embr_ab[da // 4][:, (da % 4) * R + yc * 128:
                                           (da % 4) * R + (yc + 1) * 128]
                    nc.tensor.matmul(Lp[:, 0:c], lhs, paug[da][:],
                                     start=False, stop=(da == NCH - 1))
                E = Epool.tile([128, c], F32, tag="E")
                nc.scalar.activation(out=E[:], in_=Lp[:, 0:c], func=AFT.Exp,
                                     bias=ucm_t[:, NCH:NCH + 1], scale=1.0,
                                     accum_out=ztile[:, yc:yc + 1])
                scr = scrpool.tile([128, c], F32, tag="scr")
                nc.vector.scalar_tensor_tensor(
                    out=scr[:], in0=gw_t[yc][:], scalar=1.0, in1=E[:],
                    op0=ALU.mult, op1=ALU.mult,
                    accum_out=wtile[:, yc:yc + 1])

            nc.scalar.dma_start(out=zw_d[:, 0:nr], in_=ztile[:])
            nc.gpsimd.dma_start(out=zw_d[:, nr:2 * nr], in_=wtile[:])
    nc.compile()
    _builds[key] = nc
    return nc


def _pick_pairing(n_rows: np.ndarray, n_cols: np.ndarray, r_cap: int = 256):
    """Pair 16 batches into 8 cores: row sums (m1) must fit r_cap, minimize
    the max col sum (m0) which sets the matmul free dim."""
    order = np.argsort(-n_rows, kind="stable")
    pairs = [[int(order[k]), int(order[B - 1 - k])] for k in range(B // 2)]

    def rsum(p):
        return n_rows[p[0]] + n_rows[p[1]]

    def csum(p):
        return n_cols[p[0]] + n_cols[p[1]]

    # 2-opt: shrink max col sum with swaps that keep rows under the cap
    for _ in range(64):
        worst = max(range(len(pairs)), key=lambda i: csum(pairs[i]))
        best = None
        for j in range(len(pairs)):
            if j == worst:
                continue
            for a in range(2):
                for bidx in range(2):
                    p1 = list(pairs[worst])
                    p2 = list(pairs[j])
                    p1[a], p2[bidx] = p2[bidx], p1[a]
                    if rsum(p1) > r_cap or rsum(p2) > r_cap:
                        continue
                    new_max = max(csum(p1), csum(p2))
                    if new_max < csum(pairs[worst]):
                        if best is None or new_max < best[0]:
                            best = (new_max, j, a, bidx)
        if best is None:
            break
        _, j, a, bidx = best
        pairs[worst][a], pairs[j][bidx] = pairs[j][bidx], pairs[worst][a]

    rmax = max(rsum(p) for p in pairs)
    cmax = max(csum(p) for p in pairs)
    nr = int(np.ceil(max(rmax, 1) / 128))
    c = max(144, -(-int(max(cmax, 1)) // 16) * 16)
    return nr, c, [tuple(p) for p in pairs]


def kernel(embeddings, Wq, bq, Wk, bk, attention_masks, token_type_ids):
    global LAST_RESULTS

    emb = np.ascontiguousarray(np.asarray(embeddings, dtype=np.float32))
    Wq = np.asarray(Wq, dtype=np.float32)
    Wk = np.asarray(Wk, dtype=np.float32)
    bq = np.asarray(bq, dtype=np.float32)
    bk = np.asarray(bk, dtype=np.float32)
    am = np.asarray(attention_masks)
    tt = np.asarray(token_type_ids)

    tok = am == 1
    m0 = tok & (tt == 0)   # cols
    m1 = tok & (tt == 1)   # rows
    n_cols = m0.sum(1)
    n_rows = m1.sum(1)

    nr, c, pairing = _pick_pairing(n_rows, n_cols)
    R = nr * 128
    nc = _build(nr, c)

    # ---- host constant folding
    Wq64, Wk64 = Wq.astype(np.float64), Wk.astype(np.float64)
    A = (Wq64.T @ Wk64).astype(np.float32)          # [db, da] stage-1 lhsT
    at16 = _f16(A).reshape(NCH, 128, D)
    u = (Wk64.T @ bq.astype(np.float64)).astype(np.float32)       # P bias
    ucm = np.empty((128, NCH + 1), np.float32)
    ucm[:, 0:NCH] = u.reshape(NCH, 128).T
    ucm[:, NCH] = -M0
    u2 = Wq64.T @ bk.astype(np.float64)             # prow direction
    c0 = float(bq.astype(np.float64) @ bk.astype(np.float64))

    nrm = np.sqrt(np.einsum("bsd,bsd->bs", emb, emb, dtype=np.float64))
    rr_full = (1.0 / np.maximum(nrm, EPS)).astype(np.float32)     # [B, S]

    in_maps = []
    row_meta = []
    for (b0, b1) in pairing:
        ridx = [(b, j) for b in (b0, b1) for j in np.nonzero(m1[b])[0]]
        cidx = [(b, j) for b in (b0, b1) for j in np.nonzero(m0[b])[0]]
        nrow0 = int(n_rows[b0])
        ncol0 = int(n_cols[b0])
        nrow = len(ridx)
        ncol = len(cidx)

        er = np.zeros((R, D), np.float32)
        for i, (b, j) in enumerate(ridx):
            er[i] = emb[b, j]
        ec = np.zeros((c, D), np.float32)
        for i, (b, j) in enumerate(cidx):
            ec[i] = emb[b, j]

        # pack [tok, D] -> [128, NCH*n]: chunk k at cols [k*n, (k+1)*n),
        # partition p <-> d = k*128+p
        erw = er.T.reshape(NCH, 128, R).transpose(1, 0, 2).reshape(128, NCH * R)
        ecw = ec.T.reshape(NCH, 128, c).transpose(1, 0, 2).reshape(128, NCH * c)

        prow = (ec.astype(np.float64) @ u2 + c0).astype(np.float32)
        prow[ncol:] = NEGH                      # padded cols masked via row0

        mr4 = np.zeros((4, R + c), np.float32)
        mr4[0, :R] = 1.0                        # ones row (prow + pad-col term)
        mr4[1, :nrow0] = 1.0                    # b0 rows
        mr4[2, nrow0:nrow] = 1.0                # b1 rows
        mr4[3, nrow:R] = 1.0                    # padded rows
        mr4[0, R:] = prow
        mr4[1, R + ncol0:R + ncol] = NEGH       # b1 cols, masked for b0 rows
        mr4[2, R:R + ncol0] = NEGH              # b0 cols, masked for b1 rows
        mr4[3, R:] = NEGH                       # all cols, masked for pad rows

        rrow = np.zeros((1, c), np.float32)
        rrow[0, :ncol] = [rr_full[b, j] for (b, j) in cidx]

        in_maps.append({
            "at": at16,
            "embc": _f16(ecw),
            "embr": _f16(erw),
            "mr4": _f16(mr4),
            "ucm": ucm,
            "rrow": rrow,
        })
        row_meta.append((b0, nrow0, b1, nrow - nrow0, ridx))

    valid = m0.any(axis=1) & m1.any(axis=1)
    for attempt in range(3):
        res = run_bass_kernel_spmd(nc, in_maps, core_ids=list(range(NCORES)),
                                   trace=PROFILE)
        LAST_RESULTS = res
        ok = all(np.isfinite(res.results[i]["zw"]).all() for i in range(NCORES))
        if ok:
            break
        for im in in_maps:    # overflow escape hatch: larger shift, no recompile
            im["ucm"] = im["ucm"].copy()
            im["ucm"][:, NCH] *= 4.0

    cs = np.zeros(B, np.float64)
    for i in range(NCORES):
        zw = res.results[i]["zw"].astype(np.float64)      # [128, 2*nr]
        zflat = zw[:, 0:nr].T.ravel()                     # row-major [R]
        wflat = zw[:, nr:2 * nr].T.ravel()
        b0, nrow0, b1, nrow1, ridx = row_meta[i]
        r_rows = np.zeros(R, np.float64)
        r_rows[:len(ridx)] = [rr_full[b, j] for (b, j) in ridx]
        wr = wflat * r_rows
        if valid[b0]:
            cs[b0] = wr[:nrow0].sum() / (zflat[:nrow0].sum() + 1e-300)
        if valid[b1]:
            cs[b1] = (wr[nrow0:nrow0 + nrow1].sum()
                      / (zflat[nrow0:nrow0 + nrow1].sum() + 1e-300))
    return cs.astype(np.float32)
